# revision 3
# baseline (speedup 1.0000x reference)
"""2D DCT-II (4096x4096, f32) on 8 Trainium2 NeuronCores.

out = Cm @ x @ Cn^T with Cm[u,i] = cos(pi*(2i+1)*u/(2M)) — mathematically
identical to the reference's Makhoul-FFT formulation.

Sharding: core k computes output rows [512k, 512(k+1)). Each core reads the
full x and full Cn^T from its own HBM copy; no collectives.

Dataflow per core (all fp32r matmuls, fp32 PSUM accumulate):
  step 1: A^T[c, u] = sum_i x[i, c] * CmT[i, 512k+u]     (lhsT = x tile)
  step 2: out[u, v] = sum_c A^T[c, u] * CnT[c, v]        (lhsT = A^T tile)
A^T (4096x512) and the CmT column block (4096x512) stay resident in SBUF.
"""

import sys

for _p in ("/opt/trn_rl_repo", "/opt/pypackages"):
    if _p not in sys.path:
        sys.path.append(_p)

import numpy as np

M = 4096
N = 4096
N_CORES = 8
US = M // N_CORES  # 512 output rows per core

_CACHE = {}


def _build_nc():
    import concourse.bacc as bacc
    import concourse.mybir as mybir
    from concourse import tile

    FP32R = mybir.dt.float32r
    F32 = mybir.dt.float32

    nc = bacc.Bacc("TRN2", target_bir_lowering=False, debug=False,
                   num_devices=N_CORES)
    x = nc.dram_tensor("x", [M, N], FP32R, kind="ExternalInput")
    cmt = nc.dram_tensor("cmt", [M, US], FP32R, kind="ExternalInput")
    cnt = nc.dram_tensor("cnt", [N, N], FP32R, kind="ExternalInput")
    out = nc.dram_tensor("out", [US, N], F32, kind="ExternalOutput")

    with tile.TileContext(nc) as tc:
        with (
            tc.tile_pool(name="persist", bufs=1) as persist,
            tc.tile_pool(name="stream", bufs=3) as stream,
            tc.tile_pool(name="psum", bufs=8, space="PSUM") as pp,
        ):
            # CmT column block resident in SBUF: 32 tiles [128 i, 512 u]
            cmt_sb = []
            for ig in range(32):
                t = persist.tile([128, US], FP32R, tag=f"cmt{ig}",
                                 name=f"cmt_sb{ig}")
                nc.sync.dma_start(t[:], cmt[ig * 128:(ig + 1) * 128, :])
                cmt_sb.append(t)

            # A^T cache: 32 tiles [128 c, 512 u] fp32r
            a_sb = [persist.tile([128, US], FP32R, tag=f"a{cc}",
                                 name=f"a_sb{cc}")
                    for cc in range(32)]

            # step 1: A^T[c, u] = sum_i x[i, c] cmt[i, u]
            for cg in range(8):          # 512-wide c-groups
                ps = [pp.tile([128, US], F32, tag="ps", name=f"ps1_{cg}_{i}") for i in range(4)]
                for ig in range(32):     # contraction chunks over i
                    xt = stream.tile([128, 512], FP32R, tag="xs")
                    nc.sync.dma_start(
                        xt[:], x[ig * 128:(ig + 1) * 128,
                                 cg * 512:(cg + 1) * 512])
                    for cs in range(4):
                        nc.tensor.matmul(
                            ps[cs][:],
                            xt[:, cs * 128:(cs + 1) * 128],
                            cmt_sb[ig][:],
                            start=(ig == 0), stop=(ig == 31))
                for cs in range(4):
                    nc.vector.tensor_copy(a_sb[cg * 4 + cs][:], ps[cs][:])

            # step 2: out[u, v] = sum_c A^T[c, u] cnt[c, v]
            for vg in range(8):          # 512-wide v-groups
                ps = [pp.tile([128, 512], F32, tag="ps", name=f"ps2_{vg}_{i}") for i in range(4)]
                for cgi in range(32):    # contraction chunks over c
                    ct = stream.tile([128, 512], FP32R, tag="cs")
                    nc.sync.dma_start(
                        ct[:], cnt[cgi * 128:(cgi + 1) * 128,
                                   vg * 512:(vg + 1) * 512])
                    for us in range(4):
                        nc.tensor.matmul(
                            ps[us][:],
                            a_sb[cgi][:, us * 128:(us + 1) * 128],
                            ct[:],
                            start=(cgi == 0), stop=(cgi == 31))
                for us in range(4):
                    ot = stream.tile([128, 512], F32, tag="os")
                    nc.vector.tensor_copy(ot[:], ps[us][:])
                    nc.sync.dma_start(
                        out[us * 128:(us + 1) * 128,
                            vg * 512:(vg + 1) * 512], ot[:])
    nc.finalize()
    return nc


def _dct_mat_T():
    """CmT[i, u] = cos(pi*(2i+1)*u/(2M)) as float32 (M == N here)."""
    i = np.arange(M, dtype=np.float64)
    u = np.arange(M, dtype=np.float64)
    return np.cos(np.pi * (2.0 * i[:, None] + 1.0) * u[None, :]
                  / (2.0 * M)).astype(np.float32)


def _run(x_np, trace=False):
    from concourse.bass_utils import run_bass_kernel_spmd

    if "nc" not in _CACHE:
        _CACHE["nc"] = _build_nc()
        _CACHE["ct"] = _dct_mat_T()
    nc = _CACHE["nc"]
    ct = _CACHE["ct"]

    x_np = np.ascontiguousarray(x_np, dtype=np.float32)
    in_maps = [
        {"x": x_np,
         "cmt": np.ascontiguousarray(ct[:, k * US:(k + 1) * US]),
         "cnt": ct}
        for k in range(N_CORES)
    ]
    res = run_bass_kernel_spmd(nc, in_maps, core_ids=list(range(N_CORES)),
                               trace=trace)
    out = np.concatenate([res.results[k]["out"] for k in range(N_CORES)],
                         axis=0)
    return out, res.exec_time_ns


def kernel(x):
    out, _ = _run(np.asarray(x), trace=False)
    return out
